# revision 79
# baseline (speedup 1.0000x reference)
"""GQA causal attention (ternary weights) on 8 TRN2 NeuronCores.

Strategy (tensor-parallel over heads, per sharding hint):
  - core c owns Q heads [4c, 4c+4) and KV head c.
  - host: ternarize weights; split x into THREE e4m3 fp8 levels
    (x0=fp8(x), x1=fp8(res*32), x2=fp8(res2*512)); matching fp8 weight
    copies at 1, 1/32, 1/512 (all exact e4m3 values, 2^-9 is a subnormal),
    so every projection pass is an fp8 DoubleRow matmul at natural scale.
  - device per core:
      phase 1: q/k/v projections as 3x fp8 DoubleRow passes (2 contraction
               chunks per instruction at half cost = 4x fp16 throughput)
               accumulated in fp32 PSUM. Stored as fp16 qA (q/8, row 64 =
               m~ bias), fp16 khb (k, row 64 = -1), and fp8 packs for the
               attention lo-pass: q-pack (qh/64, qres*64), k-pack
               (kres*64, kh/64). Batch-0 S~ max-pass rides these tiles;
               the cheap half of batch-1's S~ rides batch-1's proj tiles.
      phase 2: per (batch, head, q-tile): fp16 S~=QK^T in [q,k] layout for
               the row max (causal mask folded in as identity x mask-const
               matmuls on the PE; maxes via DVE X-reduce); exact S^T in
               [k,q] = fp16 hi matmul (with folded -max bias row) + one
               fp8 DoubleRow lo matmul; exp on ScalarE; PV + row sums via
               a single fp32r matmul with a 0.25 column appended to V.
               1/l via DVE reciprocal + gpsimd partition_broadcast.
      phase 3: o_proj via 2-level fp8 DoubleRow (AO0=fp8(attn/4),
               AO1=fp8(attn/4-AO0), weights 4*ternary, exact in fp8);
               fp16 partial outputs, one grouped DMA per 4 row-blocks,
               interleaved into the following q-tile's attention slots.
  - host: sum the 8 fp16 partial outputs in fp32 (row-split "all-reduce").
"""

import sys

sys.path.insert(0, "/opt/trn_rl_repo")

import numpy as np
import ml_dtypes

B = 2
S = 2048
D = 2048
NCORES = 8
HEADS_PER_CORE = 4
HD = 64
QROWS = HEADS_PER_CORE * HD  # 256
TT = 512  # token tile
MASK_NEG = -30000.0
PK_SCALE = 64.0      # S lo-pass pack scale (carrier /64, residual x64)

E4 = ml_dtypes.float8_e4m3

_CACHE = {}


def _build_program(b=B, s=S, d=D):
    import concourse.bacc as bacc
    import concourse.tile as tile
    import concourse.mybir as mybir
    from concourse import masks
    from contextlib import ExitStack

    f32 = mybir.dt.float32
    f32r = mybir.dt.float32r
    f16 = mybir.dt.float16
    f8 = mybir.dt.float8e4
    Alu = mybir.AluOpType
    Act = mybir.ActivationFunctionType
    DR = mybir.MatmulPerfMode.DoubleRow

    tokens = b * s
    n_tt = tokens // TT          # token tiles
    tt_per_b = s // TT
    n_dc = d // 128              # contraction chunks for projections
    n_qt = s // TT               # 512-wide q tiles per batch
    n_qc = s // 128              # 128-wide q chunks per batch (max pass)
    n_mt = d // 128              # output row tiles for o_proj
    n_oc = QROWS // 128          # o_proj contraction chunks (2)
    sub = TT // 128              # 128-sub-blocks per 512 tile (4)

    nc = bacc.Bacc("TRN2", target_bir_lowering=False, debug=False,
                   num_devices=NCORES)

    x_d = [nc.dram_tensor(f"x{i}", [d, tokens], f8,
                          kind="ExternalInput").ap() for i in range(3)]
    wq_d = [nc.dram_tensor(f"wq{i}", [d, QROWS], f8,
                           kind="ExternalInput").ap() for i in range(3)]
    wkv_d = [nc.dram_tensor(f"wkv{i}", [d, 128], f8,
                            kind="ExternalInput").ap() for i in range(3)]
    wo_d = nc.dram_tensor("wo8", [QROWS, d], f8, kind="ExternalInput").ap()
    out_d = nc.dram_tensor("out", [d, tokens], f16, kind="ExternalOutput").ap()

    with tile.TileContext(nc) as tc, ExitStack() as top:
        constp = top.enter_context(tc.tile_pool(name="const", bufs=1))
        wpool = top.enter_context(tc.tile_pool(name="wts", bufs=1))
        pp = top.enter_context(tc.tile_pool(name="persist", bufs=1))

        # --- constants -------------------------------------------------
        # maskKQ[p,q] = MASK_NEG where p > q (keep k<=q in [k,q] layout)
        maskKQ = constp.tile([128, 128], f16, tag="maskKQ")
        nc.gpsimd.memset(maskKQ[:], 0.0)
        nc.gpsimd.affine_select(
            out=maskKQ[:], in_=maskKQ[:], compare_op=Alu.is_ge, fill=MASK_NEG,
            base=0, pattern=[[1, 128]], channel_multiplier=-1)
        # maskQK[q,k] = MASK_NEG where k > q ([q,k] layout)
        maskQK = constp.tile([128, 128], f16, tag="maskQK")
        nc.gpsimd.memset(maskQK[:], 0.0)
        nc.gpsimd.affine_select(
            out=maskQK[:], in_=maskQK[:], compare_op=Alu.is_ge, fill=MASK_NEG,
            base=0, pattern=[[-1, 128]], channel_multiplier=1)
        identM = constp.tile([128, 128], f16, tag="identM")
        masks.make_identity(nc, identM[:])
        ident = constp.tile([128, 128], f32, tag="ident")
        masks.make_identity(nc, ident[:])


        # --- weights (DMA order tuned so tile-0 compute starts early) ---
        wq8 = [wpool.tile([128, n_dc, QROWS], f8, tag=f"wq8_{i}",
                          name=f"wq8_{i}") for i in range(3)]
        wkv8 = [wpool.tile([128, n_dc, 128], f8, tag=f"wkv8_{i}",
                           name=f"wkv8_{i}") for i in range(3)]
        nc.sync.dma_start(
            out=wq8[0][:, :, 0:128],
            in_=wq_d[0].rearrange("(c p) n -> p c n", p=128)[:, :, 0:128])

        def load_weights_rest():
            nc.sync.dma_start(
                out=wq8[0][:, :, 128:QROWS],
                in_=wq_d[0].rearrange("(c p) n -> p c n",
                                      p=128)[:, :, 128:QROWS])
            for i in range(3):
                if i > 0:
                    nc.sync.dma_start(
                        out=wq8[i][:],
                        in_=wq_d[i].rearrange("(c p) n -> p c n", p=128))
                nc.sync.dma_start(
                    out=wkv8[i][:],
                    in_=wkv_d[i].rearrange("(c p) n -> p c n", p=128))

        # o_proj weights: [128, 2(level), d] per contraction chunk; both
        # levels are the same 4*ternary data. Loaded at end of phase 1.
        wo8 = [wpool.tile([128, 2, d], f8, tag=f"wo8_{c}", name=f"wo8_{c}")
               for c in range(n_oc)]

        def load_wo8():
            for c in range(n_oc):
                for lv in range(2):
                    nc.sync.dma_start(
                        out=wo8[c][:, lv, :],
                        in_=wo_d[c * 128:(c + 1) * 128, :])

        # --- persistent activations -----------------------------------
        # qA[h]: rows 0:64 = fp16(q/8), row 64 = m~ bias (max pass)
        # qP[h]: fp8 pack [64, 2, tokens] = (qh/64, qres*64)
        # khb:   rows 0:64 = fp16(k), row 64 = -1
        # kP:    fp8 pack [64, 2, tokens] = (kres*64, kh/64)
        # vhat:  [128, chunk*65]: cols 0:64 of chunk = v, col 64 = 0.25
        qA = [pp.tile([65, tokens], f16, tag=f"qA{h}", name=f"qA{h}")
              for h in range(HEADS_PER_CORE)]
        qP = [pp.tile([64, 2, tokens], f8, tag=f"qP{h}", name=f"qP{h}")
              for h in range(HEADS_PER_CORE)]
        khb = pp.tile([65, tokens], f16, tag="khb")
        kP = pp.tile([64, 2, tokens], f8, tag="kP")
        n_ch = tokens // 128
        vhat = pp.tile([128, n_ch * 65], f32r, tag="vhat")
        nc.scalar.activation(
            vhat[:], ident[:, 0:1].to_broadcast([128, n_ch * 65]),
            Act.Copy, bias=0.25, scale=0.0)
        nc.gpsimd.memset(khb[64:65, :], -1.0)

        with ExitStack() as ph:
            mp = ph.enter_context(tc.tile_pool(name="mp", bufs=3))
            ps1 = ph.enter_context(
                tc.tile_pool(name="ps1", bufs=5, space="PSUM"))
            psst = ph.enter_context(
                tc.tile_pool(name="psst", bufs=2, space="PSUM"))
            psav = ph.enter_context(
                tc.tile_pool(name="psav", bufs=1, space="PSUM"))

            # ---------- S~ max-estimate pass, as schedulable blocks ------
            mstate = {}
            mbp = ph.enter_context(tc.tile_pool(name="mbp", bufs=8))

            def s_block(bb, h, qc):
                boff = bb * s
                if qc == 0:
                    mstate[(bb, h)] = mbp.tile([128, n_qc], f32, tag="mbuf",
                                               name="mbuf")
                mbuf = mstate[(bb, h)]
                qsl = slice(boff + qc * 128, boff + qc * 128 + 128)
                ntk = qc // sub + 1
                mtmp = mp.tile([128, 8], f32, tag="mtmp")
                for kt in range(ntk):
                    w = min(TT, (qc + 1) * 128 - kt * TT)
                    st = psst.tile([128, TT], f32, tag="st")
                    nc.tensor.matmul(
                        st[:, 0:w],
                        lhsT=qA[h][0:64, qsl],
                        rhs=khb[0:64, boff + kt * TT:boff + kt * TT + w],
                        start=True, stop=(kt != ntk - 1))
                    if kt == ntk - 1:  # diagonal block is last 128 cols
                        nc.tensor.matmul(
                            st[:, w - 128:w], lhsT=identM[:], rhs=maskQK[:],
                            start=False, stop=True, skip_group_check=True)
                    nc.vector.tensor_reduce(
                        mtmp[:, kt:kt + 1], st[:, 0:w],
                        axis=mybir.AxisListType.X, op=Alu.max)
                nc.vector.tensor_reduce(
                    mbuf[:, qc:qc + 1], mtmp[:, 0:ntk],
                    axis=mybir.AxisListType.X, op=Alu.max)

            def s_final(bb, h):
                boff = bb * s
                mbuf = mstate.pop((bb, h))
                mps = psst.tile([128, TT], f32, tag="st")
                nc.tensor.transpose(mps[0:n_qc, 0:128], mbuf[:, 0:n_qc],
                                    ident[:, 0:128])
                mrow = mp.tile([n_qc, 128], f32, tag="mrow")
                nc.scalar.copy(mrow[:], mps[0:n_qc, 0:128])
                nc.gpsimd.dma_start(
                    out=qA[h][64:65, boff:boff + s].rearrange(
                        "o (c t) -> o c t", t=128),
                    in_=mrow[:])

            # ================= phase 1: projections ====================
            ph1 = ExitStack()
            xp = ph1.enter_context(tc.tile_pool(name="xp", bufs=2))
            x8p = ph1.enter_context(tc.tile_pool(name="x8p", bufs=2))
            sp1 = ph1.enter_context(tc.tile_pool(name="sp1", bufs=3))
            for tt in range(n_tt):
                tcols = slice(tt * TT, (tt + 1) * TT)
                x8 = [x8p.tile([128, n_dc, TT], f8, tag=f"x8_{i}",
                               name=f"x8_{i}") for i in range(3)]
                for i in range(3):
                    xr = x_d[i].rearrange("(c p) t -> p c t", p=128)
                    for ch in range(2):
                        cs = slice(ch * n_dc // 2, (ch + 1) * n_dc // 2)
                        nc.sync.dma_start(
                            out=x8[i][:, cs], in_=xr[:, cs, tcols])
                if tt == 0:
                    load_weights_rest()

                def proj(w8l, mcol, ps):
                    # each 256-wide half is its own accumulation group
                    for half in range(2):
                        hs = slice(half * 256, half * 256 + 256)
                        for lv in range(3):
                            for cp in range(n_dc // 2):
                                nc.tensor.matmul(
                                    ps[:, hs],
                                    lhsT=w8l[lv][:, 2 * cp:2 * cp + 2,
                                                 mcol:mcol + 128],
                                    rhs=x8[lv][:, 2 * cp:2 * cp + 2, hs],
                                    start=(lv == 0 and cp == 0),
                                    stop=(lv == 2 and cp == n_dc // 2 - 1),
                                    perf_mode=DR,
                                    skip_group_check=(half == 1))
                                first = False

                for m in range(QROWS // 128):
                    ps = ps1.tile([128, TT], f32, tag="ps")
                    proj(wq8, m * 128, ps)
                    for i in range(2):
                        h = 2 * m + i
                        rows = slice(i * 64, i * 64 + 64)
                        # qA = fp16(q/8)
                        nc.scalar.activation(qA[h][0:64, tcols], ps[rows, :],
                                             Act.Copy, scale=0.125)
                        # res = q/8 - qA (fp16: keeps gpsimd inputs 16-bit)
                        res = sp1.tile([64, TT], f16, tag="qres")
                        nc.vector.scalar_tensor_tensor(
                            res[:], in0=ps[rows, :], scalar=0.125,
                            in1=qA[h][0:64, tcols],
                            op0=Alu.mult, op1=Alu.subtract)
                        # fp8 pack (SBUF->SBUF on gpsimd)
                        nc.gpsimd.tensor_scalar_mul(
                            qP[h][:, 0, tcols], qA[h][0:64, tcols],
                            1.0 / PK_SCALE)
                        nc.gpsimd.tensor_scalar_mul(
                            qP[h][:, 1, tcols], res[:], PK_SCALE)

                ps = ps1.tile([128, TT], f32, tag="ps")
                proj(wkv8, 0, ps)
                nc.scalar.copy(khb[0:64, tcols], ps[0:64, :])
                res = sp1.tile([64, TT], f16, tag="qres")
                nc.vector.scalar_tensor_tensor(
                    res[:], in0=ps[0:64, :], scalar=1.0,
                    in1=khb[0:64, tcols], op0=Alu.mult, op1=Alu.subtract)
                nc.gpsimd.tensor_scalar_mul(
                    kP[:, 0, tcols], res[:], PK_SCALE)
                nc.gpsimd.tensor_scalar_mul(
                    kP[:, 1, tcols], khb[0:64, tcols], 1.0 / PK_SCALE)
                vtmp = sp1.tile([64, TT], f32, tag="vtmp")
                nc.scalar.copy(vtmp[:], ps[64:128, :])
                for j in range(sub):
                    ptr = psst.tile([128, TT], f32, tag="st")
                    nc.tensor.transpose(ptr[0:128, 0:64],
                                        vtmp[:, j * 128:(j + 1) * 128],
                                        ident[0:64, 0:64])
                    ch = tt * sub + j
                    nc.scalar.copy(vhat[:, ch * 65:ch * 65 + 64],
                                   ptr[0:128, 0:64])
                # batch-0 S~ blocks for the q-chunks this tile enabled
                bb, ltt = tt // tt_per_b, tt % tt_per_b
                if bb == 0:
                    for h in range(HEADS_PER_CORE):
                        for qc in range(ltt * sub, (ltt + 1) * sub):
                            s_block(bb, h, qc)
                        if ltt == tt_per_b - 1:
                            s_final(bb, h)
                elif ltt < tt_per_b // 2:
                    # cheap half of batch-1 S~ rides the batch-1 proj tiles
                    for h in range(HEADS_PER_CORE):
                        for qc in range(ltt * sub, (ltt + 1) * sub):
                            s_block(bb, h, qc)

            ph1.close()
            # ============ phase 2 + per-batch o_proj ====================
            load_wo8()
            aop = ph.enter_context(tc.tile_pool(name="aop", bufs=1))
            ptp = ph.enter_context(tc.tile_pool(name="ptp", bufs=6))
            outp = ph.enter_context(tc.tile_pool(name="outp", bufs=3))
            # ao[c]: fp8 pack [128, 2(level), tokens]
            ao = [aop.tile([128, 2, tokens], f8, tag=f"ao{i}", name=f"ao{i}")
                  for i in range(n_oc)]

            def av(pav, pt, lo, w, bb, kc, nchunks):
                ch = bb * (s // 128) + kc
                nc.tensor.matmul(
                    pav[:, lo:lo + w], lhsT=vhat[:, ch * 65:ch * 65 + 65],
                    rhs=pt[:, lo:lo + w],
                    start=(kc == 0), stop=(kc == nchunks - 1),
                    skip_group_check=True)

            b1q = []
            for h in range(HEADS_PER_CORE):
                for qc in range(n_qc // 2, n_qc):
                    b1q.append(lambda h=h, qc=qc: s_block(1, h, qc))
                b1q.append(lambda h=h: s_final(1, h))
            # weight S~ pacing by main-slot size (qt+1 chunks of work);
            # slots run qt-major
            wsum = HEADS_PER_CORE * n_qt * (n_qt + 1) // 2
            bcum, acc = [], 0.0
            for qt in range(n_qt):
                for h in range(HEADS_PER_CORE):
                    acc += (qt + 1) * len(b1q) / wsum
                    bcum.append(min(int(round(acc)), len(b1q)))
            bcum[-1] = len(b1q)

            out_r = out_d.rearrange("(mm p) t -> p mm t", p=128)

            def oproj_group(bb, qt, g, eng):
                """4 consecutive m-blocks of one q-tile, one DMA out."""
                boff = bb * s
                osb = outp.tile([128, 4, TT], f16, tag="og", name="osbg")
                for mi in range(4):
                    m = g * 4 + mi
                    po = ps1.tile([128, TT], f32, tag="ps")
                    for half in range(2):
                        hs = slice(half * 256, half * 256 + 256)
                        qsl = slice(boff + qt * TT + half * 256,
                                    boff + qt * TT + half * 256 + 256)
                        for ci in range(n_oc):
                            nc.tensor.matmul(
                                po[:, hs],
                                lhsT=wo8[ci][:, :, m * 128:m * 128 + 128],
                                rhs=ao[ci][:, :, qsl],
                                start=(ci == 0), stop=(ci == n_oc - 1),
                                perf_mode=DR,
                                skip_group_check=(half == 1))
                    dst = osb[:, mi, :]
                    if eng == "v":
                        nc.vector.tensor_copy(dst, po[:])
                    else:
                        nc.scalar.copy(dst, po[:])
                nc.sync.dma_start(
                    out=out_r[:, g * 4:g * 4 + 4,
                              boff + qt * TT:boff + (qt + 1) * TT],
                    in_=osb[:])

            def attn_slot(bb, h, qt):
                boff = bb * s
                qlo = boff + qt * TT
                pav = psav.tile([65, TT], f32, tag="pav")
                nchunks = (qt + 1) * sub
                pipe = []
                for kc in range(nchunks):
                    ksl = slice(boff + kc * 128, boff + kc * 128 + 128)
                    j = kc - qt * sub
                    lo = max(j, 0) * 128  # cols < lo fully masked
                    w = TT - lo
                    s2 = ps1.tile([128, TT], f32, tag="ps")
                    nc.tensor.matmul(
                        s2[:, lo:lo + w], lhsT=khb[:, ksl],
                        rhs=qA[h][:, qlo + lo:qlo + TT],
                        start=True, stop=False)
                    # fp8 DoubleRow lo-pass (<=256-wide halves)
                    nhalf = (w + 255) // 256
                    for hf in range(nhalf):
                        hlo = lo + hf * 256
                        hw = min(256, TT - hlo)
                        nc.tensor.matmul(
                            s2[:, hlo:hlo + hw],
                            lhsT=kP[:, :, ksl],
                            rhs=qP[h][:, :, qlo + hlo:qlo + hlo + hw],
                            start=False,
                            stop=(j < 0 and hf == nhalf - 1),
                            perf_mode=DR, skip_group_check=True)
                    if j >= 0:
                        nc.tensor.matmul(
                            s2[:, lo:lo + 128], lhsT=identM[:],
                            rhs=maskKQ[:], start=False, stop=True,
                            skip_group_check=True)
                    pt = ptp.tile([128, TT], f32r, tag="pt")
                    nc.scalar.activation(pt[:, lo:lo + w],
                                         s2[:, lo:lo + w], Act.Exp)
                    pipe.append((pt, lo, w, kc))
                    if len(pipe) > 3:
                        pv = pipe.pop(0)
                        av(pav, pv[0], pv[1], pv[2], bb, pv[3], nchunks)
                for pv in pipe:
                    av(pav, pv[0], pv[1], pv[2], bb, pv[3], nchunks)
                pipe.clear()

                # pav row 64 = l/4; rec = 4/l; broadcast on gpsimd; then
                # aof = PV * 4/l = 4*attn (fp16). AO0 = fp8(aof/16)
                # = fp8(attn/4), AO1 = fp8(aof/16 - AO0) = attn/4 - AO0.
                # Both wo8 levels are 4*ternary.
                # Pool only sees fp16->fp8 ops; the mixed stt runs on DVE.
                rec = mp.tile([1, TT], f32, tag="rec")
                with nc.allow_low_precision(
                        reason="1/l broadcast feeds fp8 conversions"):
                    nc.vector.reciprocal(rec[:], pav[64:65, :])
                bcs = mp.tile([64, TT], f32, tag="bcs")
                nc.gpsimd.partition_broadcast(bcs[:], rec[:])
                # aof half matches ao's base partition (SB+SB ops
                # require equal base partitions)
                rows = slice((h % 2) * 64, (h % 2) * 64 + 64)
                aof = mp.tile([128, TT], f16, tag="aof")
                nc.vector.tensor_tensor(
                    aof[rows, :], pav[0:64, :], bcs[:], op=Alu.mult)
                c = h // 2
                nc.gpsimd.tensor_scalar_mul(
                    ao[c][rows, 0, qlo:qlo + TT], aof[rows, :], 1.0 / 16)
                nc.vector.scalar_tensor_tensor(
                    ao[c][rows, 1, qlo:qlo + TT],
                    in0=aof[rows, :], scalar=1.0 / 16,
                    in1=ao[c][rows, 0, qlo:qlo + TT],
                    op0=Alu.mult, op1=Alu.subtract)

            # Both batches qt-major: each q-tile's o_proj rides the next
            # q-tile's attention slots. Batch-0 slots also carry the
            # batch-1 S~ rideshare; batch-0's o_proj copies go to ScalarE
            # (DVE is reduce-bound there), batch-1's to DVE (ScalarE is
            # exp-bound there).
            slot = 0
            for qt in range(n_qt):
                for h in range(HEADS_PER_CORE):
                    attn_slot(0, h, qt)
                    lo_i = bcum[slot - 1] if slot else 0
                    for fn in b1q[lo_i:bcum[slot]]:
                        fn()
                    slot += 1
                    if qt > 0:
                        oproj_group(0, qt - 1, h, "s")
            # pending o_proj groups pop one per slot, delayed one slot so
            # the group's ao dependencies never head-block the PE queue
            pend = [(0, n_qt - 1, g, "v") for g in range(HEADS_PER_CORE)]
            for qt in range(n_qt):
                for h in range(HEADS_PER_CORE):
                    attn_slot(1, h, qt)
                    if qt > 0 or h > 0:
                        oproj_group(*pend.pop(0))
                pend += [(1, qt, g, "v") for g in range(HEADS_PER_CORE)]
            for args in pend:
                oproj_group(*args)

    nc.compile()
    return nc


def _ternarize(w):
    w = np.asarray(w, np.float32)
    scale = max(np.abs(w).mean(), 1e-6)
    return ((w > 0.05 * scale).astype(np.float32)
            - (w < -0.05 * scale).astype(np.float32))


def kernel(x, wq, wk, wv, wo):
    from concourse.bass_utils import run_bass_kernel_spmd

    if "nc" not in _CACHE:
        _CACHE["nc"] = _build_program()
    nc = _CACHE["nc"]

    tq = _ternarize(wq)
    tk = _ternarize(wk)
    tv = _ternarize(wv)
    to = _ternarize(wo)

    xT = np.ascontiguousarray(np.asarray(x, np.float32).reshape(B * S, D).T)
    # 3-level e4m3 split of x; weight copies at 1, 1/32, 1/512 (all exact)
    x0 = xT.astype(E4)
    r1 = xT - x0.astype(np.float32)
    x1 = (r1 * 32.0).astype(E4)
    r2 = r1 - x1.astype(np.float32) / 32.0
    x2 = (r2 * 512.0).astype(E4)
    xs = [x0, x1, x2]
    scales = [1.0, 1.0 / 32, 1.0 / 512]

    in_maps = []
    for c in range(NCORES):
        qsl = slice(c * QROWS, (c + 1) * QROWS)
        ksl = slice(c * HD, (c + 1) * HD)
        wkv = np.concatenate([tk[ksl], tv[ksl]], axis=0)  # [128, D]
        wqT = np.ascontiguousarray(tq[qsl].T)
        wkvT = np.ascontiguousarray(wkv.T)
        m = {"wo8": np.ascontiguousarray(to[:, qsl].T * 4.0).astype(E4)}
        for i in range(3):
            m[f"x{i}"] = xs[i]
            m[f"wq{i}"] = (wqT * scales[i]).astype(E4)
            m[f"wkv{i}"] = (wkvT * scales[i]).astype(E4)
        in_maps.append(m)

    res = run_bass_kernel_spmd(nc, in_maps, list(range(NCORES)))
    total = res.results[0]["out"].astype(np.float32)
    for c in range(1, NCORES):
        total = total + res.results[c]["out"].astype(np.float32)
    return np.ascontiguousarray(total.T).reshape(B, S, D).astype(np.float32)
